# revision 45
# baseline (speedup 1.0000x reference)
"""Trainium2 Bass kernel for a GNN message-passing layer.

Reference semantics (per edge e = (src j, dst i)):
    m_in  = [x_j, pos_j - pos_i]                 # [E, 6]
    h     = celu(m_in @ f_w1 + f_b1)             # [E, 64]
    msg   = relu(h @ f_w2 + f_b2)                # [E, 64]
    aggr  = segment_max(msg, dst, N); empty -> 0 # [N, 64]
    u     = celu([aggr, x] @ g_w1 + g_b1)
    out   = celu(u @ g_w2 + g_b2)                # [N, 64]

Sharding: nodes are split into 8 contiguous ranges (6250 per core); each core
receives exactly the edges whose dst lands in its range, so the segment-max is
purely local (no collective).  The host does index-only work: it sorts each
core's nodes by in-degree, lays edges out in "rounds" (round r = the r-th edge
of every node that has one), pairs rounds two-high into 128-partition tiles,
and pads with duplicate edges (max is idempotent).  The device then does every
FLOP: the per-edge MLP as three accumulated matmul streams (celu decomposed as
celu(z) = z + relu(-z) + exp(-relu(-z)) - 1, with the linear z term re-streamed
through the combined weight W1@W2 and all constants folded into one bias), a
running tensor_max over the round tiles, and the node MLP with the same trick.
"""

import math
import os
import sys

import numpy as np

N = 50000
E = 1600000
CORES = 8
NCN = N // CORES            # nodes per core
TILE = 512                  # fp32 matmul moving free dim / one PSUM bank
GRP = 1024                  # uniform processing-group width (columns)
NCW = ((NCN + GRP - 1) // GRP) * GRP      # aggr width per core (7168)
SUP = 4096                  # feats DMA staging superblock (columns)
F32 = np.float32


# --------------------------------------------------------------------------
# host-side layout (index work only)
# --------------------------------------------------------------------------

def _core_layouts(edge_index):
    """Per-core node ordering + degree-sorted CSR of local edges."""
    dst = np.asarray(edge_index[1])
    cores = []
    for c in range(CORES):
        lo, hi = c * NCN, (c + 1) * NCN
        eids = np.nonzero((dst >= lo) & (dst < hi))[0]
        ldst = (dst[eids] - lo).astype(np.int64)
        deg = np.bincount(ldst, minlength=NCN)
        order = np.argsort(-deg, kind="stable")         # node ranks
        rank = np.empty(NCN, np.int64)
        rank[order] = np.arange(NCN)
        perm = np.argsort(rank[ldst], kind="stable")
        es = eids[perm]                                  # edges sorted by rank
        deg_s = deg[order]
        row_start = np.zeros(NCN + 1, np.int64)
        np.cumsum(deg_s, out=row_start[1:])
        cores.append(dict(es=es, deg_s=deg_s, row_start=row_start,
                          order=order, empty=order[deg_s == 0] + lo))
    return cores


def _tile_plan(cores):
    """Shared (SPMD-uniform) tile plan.

    All groups are uniform GRP (=1024) columns: matmul instructions carry at
    most ONE hardware sync wait, so every group needs "twin" matmuls with a
    free wait slot for redistributed dependencies (see _build_nc).

    Returns (tiles, groups, S):
      tiles  : list of (pair_round t, k) -- k-th 512-tile of pair-round t
      groups : list of (slot_col0, aggr_col0, fd, is_first_round)
      S      : total slot columns (= 512 * len(tiles))
    """
    rmax = max(int(c["deg_s"][0]) for c in cores)
    n_pairs = (rmax + 1) // 2
    tiles = []
    for t in range(n_pairs):
        w = max(int(np.searchsorted(-c["deg_s"], -(2 * t), side="left"))
                for c in cores)      # max over cores of #nodes with deg > 2t
        k_t = 2 * max(1, (w + GRP - 1) // GRP)       # 512-tiles, even count
        for k in range(k_t):
            tiles.append((t, k))
    S = TILE * len(tiles)

    groups = []
    for i in range(0, len(tiles), 2):
        t, k = tiles[i]
        groups.append((i * TILE, k * TILE, GRP, t == 0))
    return tiles, groups, S


def _pack_core(core, tiles, S, x, pos, src, dst):
    """Build one core's slot->edge assignment and gather features."""
    es, deg_s, row_start = core["es"], core["deg_s"], core["row_start"]
    ncols = len(tiles) * TILE
    nvec = np.tile(np.arange(TILE, dtype=np.int64), len(tiles))  # col in tile
    kvec = np.repeat([k for (_, k) in tiles], TILE)
    tvec = np.repeat([t for (t, _) in tiles], TILE)
    node = kvec * TILE + nvec                    # node rank targeted by column

    safe_node = np.minimum(node, NCN - 1)
    ecap = len(es) - 1
    first_edge = es[np.minimum(row_start[safe_node], ecap)]  # dup fallback
    # nodes with deg 0 or node>=NCN: fall back to edge es[0] (results ignored
    # or fixed up on host)
    bad = (node >= NCN) | (deg_s[safe_node] == 0)
    first_edge = np.where(bad, es[0], first_edge)

    def round_edges(r):
        has = (~bad) & (deg_s[safe_node] > r)
        idx = np.minimum(row_start[safe_node] + np.where(has, r, 0), ecap)
        return np.where(has, es[idx], first_edge)

    a_e = round_edges(2 * tvec)        # vectorized: r differs per column
    b_e = round_edges(2 * tvec + 1)

    feats = np.empty((18, S), dtype=F32)
    for half, eids in ((0, a_e), (9, b_e)):
        s, d = src[eids], dst[eids]
        feats[half + 0:half + 3, :ncols] = x[s].T
        feats[half + 3:half + 6, :ncols] = pos[s].T
        feats[half + 6:half + 9, :ncols] = pos[d].T
    if ncols < S:
        feats[:, ncols:] = 0.0

    xnode = np.zeros((3, NCW), dtype=F32)
    xnode[:, :NCN] = x[core["order"] + 0].T      # caller adds core offset
    return feats, xnode


# column layout of the packed weights tensor [128, WCOL]
WSLOTS = dict(w2=(128, 0, 128), w1n=(18, 128, 128), w12=(18, 256, 128),
              g1n=(67, 384, 64), g12=(67, 448, 64), g2=(64, 512, 64),
              nbias1=(128, 576, 1), cbias=(64, 577, 1), nbias_g1=(64, 578, 1),
              nbias_gf=(64, 579, 1), pbias_gf=(64, 580, 1))
WCOL = 584


def _weights(f_w1, f_b1, f_w2, f_b2, g_w1, g_b1, g_w2, g_b2):
    w9 = np.concatenate([f_w1[0:3], f_w1[3:6], -f_w1[3:6]], axis=0)  # [9,64]
    blk = lambda m: np.block([[m, np.zeros_like(m)], [np.zeros_like(m), m]])
    w12 = (w9 @ f_w2).astype(F32)
    cbias = (f_b1 @ f_w2 - f_w2.sum(axis=0) + f_b2).astype(F32)       # [64]
    gbias = (g_b1 @ g_w2 - g_w2.sum(axis=0) + g_b2).astype(F32)       # [64]
    w = dict(
        w1n=blk(-w9).astype(F32),            # [18,128]
        w12=blk(w12),                        # [18,128]
        w2=blk(f_w2).astype(F32),            # [128,128]
        nbias1=np.tile(-f_b1, 2).astype(F32).reshape(128, 1),
        cbias=cbias.reshape(64, 1),
        g1n=(-g_w1).astype(F32),             # [67,64]
        g12=(g_w1 @ g_w2).astype(F32),       # [67,64]
        g2=g_w2.astype(F32),                 # [64,64]
        nbias_g1=(-g_b1).astype(F32).reshape(64, 1),
        nbias_gf=(-gbias).reshape(64, 1),
        pbias_gf=gbias.reshape(64, 1),
    )
    wpack = np.zeros((128, WCOL), dtype=F32)
    for name, (p, c0, cn) in WSLOTS.items():
        wpack[:p, c0:c0 + cn] = w[name]
    w["wpack"] = wpack
    return w


# --------------------------------------------------------------------------
# numpy model of the device program (for validation)
# --------------------------------------------------------------------------

def _numpy_device(feats, xnode, w, groups):
    aggr = np.empty((128, NCW), dtype=F32)
    for (c0, a0, fd, first) in groups:
        f = feats[:, c0:c0 + fd]
        zb = w["w1n"].T.astype(F32) @ f
        msg = w["w12"].T @ f
        r = np.maximum(zb + w["nbias1"], 0).astype(F32)
        e = np.exp(-r).astype(F32)
        msg = msg + w["w2"].T @ r + w["w2"].T @ e
        if first:
            aggr[:, a0:a0 + fd] = msg
        else:
            aggr[:, a0:a0 + fd] = np.maximum(aggr[:, a0:a0 + fd], msg)
    a64 = np.maximum(aggr[0:64], aggr[64:128])
    u_in = np.empty((67, NCW), dtype=F32)
    u_in[0:64] = np.maximum(a64 + w["cbias"], 0)
    u_in[64:67] = xnode
    zg = w["g1n"].T @ u_in
    rg = np.maximum(zg + w["nbias_g1"], 0).astype(F32)
    eg = np.exp(-rg).astype(F32)
    o2 = w["g12"].T @ u_in + w["g2"].T @ rg + w["g2"].T @ eg
    rf = np.maximum(-o2 + w["nbias_gf"], 0).astype(F32)
    ef = np.exp(-rf).astype(F32)
    vf = np.maximum(o2 + w["pbias_gf"], 0).astype(F32)
    return (vf - 1.0 + ef).astype(F32)        # [64, NCW]


# --------------------------------------------------------------------------
# bass program
# --------------------------------------------------------------------------

def _import_concourse():
    try:
        import concourse.bass  # noqa: F401
    except ImportError:
        sys.path.insert(0, "/opt/trn_rl_repo")


def _install_ntff_shim():
    """Provide antenv.axon_hooks (missing in this image) so that
    run_bass_kernel_spmd(trace=True) can capture NTFF profiles through
    libaxon's C ABI (same mechanism as trn_boot's degraded hook)."""
    import contextlib
    import ctypes
    import types

    if "antenv.axon_hooks" in sys.modules:
        return
    so_path = "/opt/axon/libaxon_pjrt.so"
    if not os.path.exists(so_path):
        return
    lib = ctypes.CDLL(so_path)
    if not hasattr(lib, "axon_start_nrt_profile"):
        return
    lib.axon_start_nrt_profile.argtypes = [ctypes.POINTER(ctypes.c_int64),
                                           ctypes.c_size_t]
    lib.axon_start_nrt_profile.restype = ctypes.c_int64
    lib.axon_stop_nrt_profile.argtypes = [ctypes.c_char_p]
    lib.axon_stop_nrt_profile.restype = ctypes.c_int64

    @contextlib.contextmanager
    def _hook(output_dir, device_ids):
        import jax
        jax.devices()
        if device_ids:
            ids = (ctypes.c_int64 * len(device_ids))(*device_ids)
            rc = lib.axon_start_nrt_profile(ids, len(device_ids))
        else:
            rc = lib.axon_start_nrt_profile(None, 0)
        if rc != 0:
            raise RuntimeError(f"axon_start_nrt_profile rc={rc}")
        try:
            yield
        finally:
            n = lib.axon_stop_nrt_profile(str(output_dir).encode())
            print(f"ntff profile: {n} file(s) -> {output_dir}",
                  file=sys.stderr)

    mod = types.ModuleType("antenv.axon_hooks")
    mod.get_axon_ntff_profile_hook = lambda: _hook
    mod.set_axon_ntff_profile_hook = lambda h: None
    sys.modules["antenv.axon_hooks"] = mod


def _dep(from_inst, to_inst, reason):
    from concourse.tile import add_dep_helper
    a = getattr(from_inst, "ins", from_inst)
    b = getattr(to_inst, "ins", to_inst)
    add_dep_helper(a, b, reason=reason)


def _build_nc(groups, S):
    _import_concourse()
    import concourse.bass as bass
    import concourse.tile as tile
    import concourse.tile_sem_assignment as _tsa
    from concourse import mybir

    # One DMAHW bookkeeping lane: HWDGE transfers then share a FIFO proc, so
    # DMA-vs-DMA ordering (slot WAW) needs no extra sync wait — ISA structs
    # carry at most one wait each.
    _tsa.NUM_HWDGE_SEMS = 1

    f32 = mybir.dt.float32
    AF = mybir.ActivationFunctionType
    nc = bass.Bass()

    feats_d = nc.dram_tensor("feats", [18, S], f32, kind="ExternalInput")
    xnode_d = nc.dram_tensor("xnode", [3, NCW], f32, kind="ExternalInput")
    wpack_d = nc.dram_tensor("wpack", [128, WCOL], f32, kind="ExternalInput")
    out_d = nc.dram_tensor("out", [64, NCW], f32, kind="ExternalOutput")

    n_sup = (S + SUP - 1) // SUP

    with tile.TileContext(nc) as tc:
        with (
            tc.tile_pool(name="const", bufs=1) as cpool,
            tc.tile_pool(name="aggr", bufs=1) as apool,
            tc.tile_pool(name="feats", bufs=2) as fpool,
            tc.tile_pool(name="re", bufs=2) as repool,
            tc.tile_pool(name="gwork", bufs=1) as gpool,
            tc.tile_pool(name="psum_z", bufs=2, space="PSUM") as pz,
            tc.tile_pool(name="psum_m", bufs=2, space="PSUM") as pm,
        ):
            wsb = cpool.tile([128, WCOL], f32, name="wsb")
            wdma = nc.sync.dma_start(wsb[:], wpack_d[:])
            w = {name: wsb[0:p, c0:c0 + cn]
                 for name, (p, c0, cn) in WSLOTS.items()}
            # ACT-side absorber: observe the weights DMA once so the first
            # bias-consuming activation doesn't need a second wait.
            tabs = cpool.tile([1, 1], f32, name="tabs")
            nc.scalar.activation(tabs[:], wsb[0:1, 0:1], AF.Copy)

            aggr = apool.tile([128, NCW], f32)

            # Matmult instructions can carry exactly one hardware sync wait;
            # a tiny absorber matmul observes the weights DMA so later
            # matmuls never need a second wait for it.
            scratch = pz.tile([128, GRP], f32, tag="zb", name="scratch")
            nc.tensor.matmul(scratch[0:1, 0:1], wsb[0:1, 0:1], wsb[0:1, 0:1],
                             start=True, stop=True)

            # Wait-absorber micro-ops: every ISA struct carries at most ONE
            # sync wait, so secondary dependencies are pre-observed by tiny
            # ops on the same engine/queue, ordered before the real op.
            vscr = cpool.tile([1, len(groups) + 4], f32, name="vscr")
            ascr = cpool.tile([1, NCW // TILE + 2], f32, name="ascr")
            ascr2 = cpool.tile([1, NCW // TILE + 2], f32, name="ascr2")
            # DVE-side absorber: observe the weights DMA once so DVE micro-
            # copies sourced from wsb need no DMA wait of their own.
            tvd0 = nc.vector.tensor_copy(vscr[0:1, len(groups) + 1:
                                              len(groups) + 2], wsb[0:1, 0:1])
            _dep(tvd0, wdma, "DVE observes weights DMA")

            sup_tiles = []
            sup_dmas = []
            for i in range(n_sup):
                cols = min(SUP, S - i * SUP)
                st = fpool.tile([18, SUP], f32, tag="feats_sup")
                d = nc.sync.dma_start(st[:, :cols],
                                      feats_d[:, i * SUP:i * SUP + cols])
                sup_tiles.append(st)
                sup_dmas.append(d)

            runmax = []          # per-group reducer instruction
            for gi, (c0, a0, fd, first) in enumerate(groups):
                st = sup_tiles[c0 // SUP]
                fo = c0 % SUP
                fa = st[:, fo:fo + fd]
                zb = pz.tile([128, fd], f32, tag="zb")
                ms = pm.tile([128, fd], f32, tag="ms")
                mm_zb = [nc.tensor.matmul(zb[:, o:o + TILE], w["w1n"],
                                          fa[:, o:o + TILE],
                                          start=True, stop=True)
                         for o in range(0, fd, TILE)]
                # redistribute waits: the DVE release of this group's ms slot
                # lands on the second zb matmul (wait-free) instead of the
                # first ms matmul (which already carries a PE self-wait).
                if gi >= 2:
                    _dep(mm_zb[1], runmax[gi - 2], "ms-slot release via zb twin")
                # a new feats superblock must land before the NEXT group that
                # reads it; its wait goes on this group's e-twin (below).
                for o in range(0, fd, TILE):
                    nc.tensor.matmul(ms[:, o:o + TILE], w["w12"],
                                     fa[:, o:o + TILE], start=True, stop=False)
                r = repool.tile([128, fd], f32, tag="r")
                e = repool.tile([128, fd], f32, tag="e")
                nc.scalar.activation(r[:], zb[:], AF.Relu,
                                     bias=w["nbias1"], scale=1.0)
                nc.scalar.activation(e[:], r[:], AF.Exp, scale=-1.0)
                for o in range(0, fd, TILE):
                    nc.tensor.matmul(ms[:, o:o + TILE], w["w2"],
                                     r[:, o:o + TILE], start=False, stop=False)
                mm_e = [nc.tensor.matmul(ms[:, o:o + TILE], w["w2"],
                                         e[:, o:o + TILE],
                                         start=False, stop=(o + TILE >= fd))
                        for o in range(0, fd, TILE)]
                nxt = (c0 + fd) // SUP
                if nxt > c0 // SUP and nxt < n_sup:
                    _dep(mm_e[1], sup_dmas[nxt], "sup prefetch via e twin")
                # DVE pre-observes the msg matmuls' completion so the reducer
                # carries only its own in-order RAW wait.
                tv = nc.vector.tensor_copy(vscr[0:1, gi:gi + 1],
                                           wsb[0:1, 0:1])
                _dep(tv, mm_e[1], "absorb reducer PE wait")
                dst_ap = aggr[:, a0:a0 + fd]
                if first:
                    rm = nc.vector.tensor_copy(dst_ap, ms[:])
                else:
                    rm = nc.vector.tensor_max(dst_ap, dst_ap, ms[:])
                _dep(rm, tv, "order after absorber")
                runmax.append(rm)
                last_mm = mm_e[1]
                zb_last = zb

            # ---- node phase ----
            # TensorTensor needs equal base partitions for SBUF inputs:
            # DMA-move the odd-rounds half (partitions 64-127) down to 0-63.
            ah = gpool.tile([64, NCW], f32, tag="ah")
            ahdma = nc.sync.dma_start(ah[:], aggr[64:128, :])
            tva = nc.vector.tensor_copy(vscr[0:1, len(groups):len(groups) + 1],
                                        wsb[0:1, 0:1])
            _dep(tva, ahdma, "absorb aggr-move DMA wait")
            fold = nc.vector.tensor_max(ah[:], aggr[0:64, :], ah[:])
            _dep(fold, tva, "order after absorber")
            u_in = gpool.tile([67, NCW], f32, tag="u_in")
            urelu = nc.scalar.activation(u_in[0:64, :], ah[:], AF.Relu,
                                         bias=w["cbias"], scale=1.0)
            xdma = nc.sync.dma_start(u_in[64:67, :], xnode_d[:])
            out_sb = gpool.tile([64, NCW], f32, tag="out_sb")

            # Absorber chain: tiny matmuls into the last group's dead zb
            # tile (claiming no new PSUM slot) make PE observe the final
            # reducer's DVE tick, the xnode DMA, and the u_in relu, so each
            # g-phase matmul keeps at most one hardware wait (its own PSUM
            # slot-reuse self-wait).
            scr2 = zb_last
            t2 = nc.tensor.matmul(scr2[0:1, 0:1], wsb[0:1, 0:1],
                                  wsb[0:1, 0:1], start=True, stop=False)
            _dep(t2, runmax[-1], "observe final reducer DVE tick")
            t3 = nc.tensor.matmul(scr2[0:1, 0:1], wsb[0:1, 0:1],
                                  wsb[0:1, 0:1], start=False, stop=False)
            _dep(t3, xdma, "observe xnode DMA")
            t4 = nc.tensor.matmul(scr2[0:1, 0:1], wsb[0:1, 0:1],
                                  wsb[0:1, 0:1], start=False, stop=True)
            _dep(t4, urelu, "observe u_in relu ACT tick")

            for i in range(NCW // TILE):
                ui = u_in[:, i * TILE:(i + 1) * TILE]
                zg = pz.tile([64, TILE], f32, tag="zb")
                o2 = pm.tile([64, TILE], f32, tag="ms")
                mm_zg = nc.tensor.matmul(zg[:], w["g1n"], ui,
                                         start=True, stop=True)
                nc.tensor.matmul(o2[:], w["g12"], ui, start=True, stop=False)
                rg = repool.tile([64, TILE], f32, tag="r")
                eg = repool.tile([64, TILE], f32, tag="e")
                # ACT pre-observes the g1 matmul so rg keeps only its own
                # slot-WAW wait
                tag_ = nc.scalar.activation(ascr2[0:1, i:i + 1], wsb[0:1, 0:1],
                                            AF.Copy)
                _dep(tag_, mm_zg, "absorb rg PE wait")
                rgi = nc.scalar.activation(rg[:], zg[:], AF.Relu,
                                           bias=w["nbias_g1"], scale=1.0)
                _dep(rgi, tag_, "order after absorber")
                nc.scalar.activation(eg[:], rg[:], AF.Exp, scale=-1.0)
                nc.tensor.matmul(o2[:], w["g2"], rg[:], start=False,
                                 stop=False)
                nc.tensor.matmul(o2[:], w["g2"], eg[:], start=False,
                                 stop=True)
                rf = repool.tile([64, TILE], f32, tag="rf")
                ef = repool.tile([64, TILE], f32, tag="ef")
                vf = repool.tile([64, TILE], f32, tag="vf")
                rf_act_deps = []
                if i >= 2:
                    # ACT pre-observes the combiner's DVE tick (releases the
                    # rf/ef/vf slots of tile i-2)
                    ta = nc.scalar.activation(ascr[0:1, i:i + 1],
                                              wsb[0:1, 0:1], AF.Copy)
                    _dep(ta, stt_prev2, "absorb final-combine DVE wait")
                    rf_act_deps.append(ta)
                rfi = nc.scalar.activation(rf[:], o2[:], AF.Relu,
                                           bias=w["nbias_gf"], scale=-1.0)
                for ta_ in rf_act_deps:
                    _dep(rfi, ta_, "order after absorber")
                nc.scalar.activation(ef[:], rf[:], AF.Exp, scale=-1.0)
                nc.scalar.activation(vf[:], o2[:], AF.Relu,
                                     bias=w["pbias_gf"], scale=1.0)
                stt = nc.vector.scalar_tensor_tensor(
                    out_sb[:, i * TILE:(i + 1) * TILE], vf[:], -1.0, ef[:],
                    op0=mybir.AluOpType.add, op1=mybir.AluOpType.add)
                if i >= 1:
                    stt_prev2 = stt_prev
                stt_prev = stt

            nc.sync.dma_start(out_d[:], out_sb[:])

    _prune_waits(nc)
    return nc


def _prune_waits(nc):
    """ISA structs carry at most one sync wait. Drop provably-redundant
    waits Tile emitted:

    1. same-engine self-waits on compute instructions other than Matmult:
       ACT/DVE/Pool queues are strict FIFO and each op fully drains before
       the next issues, so an earlier instruction on the same engine is
       always complete; the dependency the wait encodes is enforced by
       program order (the earlier instruction itself blocks the queue while
       ITS waits are pending).  PE kept: consecutive matmuls overlap
       fill/drain in the array.
    2. DMA-vs-DMA ordering waits on transfers that also carry a compute
       wait: in this program's dataflow the compute dependency is on
       readers of the slot's previous contents (or on consumers downstream
       of every earlier conflicting transfer), and a completed read implies
       the producing DMA completed.
    """
    n1 = n2 = 0
    for b in nc.m.functions[0].blocks:
        for i in b.instructions:
            si = i.sync_info
            if si is None or not si.on_wait or len(si.on_wait) < 2:
                continue
            nm = type(i).__name__
            waits = list(si.on_wait)
            if nm == "InstDrain":
                # kernel-tail drain: every engine's last instruction is
                # observed (transitively) by the final output DMA, so the
                # single DMAHW wait subsumes the engine waits here.
                dma_w = [x for x in waits if x.ant_name.startswith("DMAHW")]
                if dma_w:
                    si.on_wait = dma_w[-1:]
                else:
                    si.on_wait = waits[-1:]
                continue
            if nm == "InstDMACopy":
                if any(not x.ant_name.startswith("DMAHW") and
                       not x.ant_name.startswith("DMASW") for x in waits):
                    kept = [x for x in waits
                            if not (x.ant_name.startswith("DMAHW") or
                                    x.ant_name.startswith("DMASW"))]
                    n2 += len(waits) - len(kept)
                    waits = kept
            elif nm != "InstMatmult":
                own = str(i.engine).split(".")[-1]
                kept = [x for x in waits
                        if x.ant_name.rsplit("_", 1)[0] != own]
                if len(kept) < len(waits):
                    n1 += len(waits) - len(kept)
                    waits = kept
            si.on_wait = waits
    return n1, n2


# --------------------------------------------------------------------------
# entry points
# --------------------------------------------------------------------------

def _prepare(x, pos, edge_index, f_w1, f_b1, f_w2, f_b2,
             g_w1, g_b1, g_w2, g_b2):
    x = np.asarray(x, F32)
    pos = np.asarray(pos, F32)
    src = np.asarray(edge_index[0]).astype(np.int64)
    dst = np.asarray(edge_index[1]).astype(np.int64)
    cores = _core_layouts(edge_index)
    tiles, groups, S = _tile_plan(cores)
    S_pad = ((S + SUP - 1) // SUP) * SUP
    packs = []
    for c, core in enumerate(cores):
        feats, xnode = _pack_core(core, tiles, S_pad, x, pos, src, dst)
        xnode[:, :NCN] = x[core["order"] + c * NCN].T
        packs.append((feats, xnode))
    w = _weights(np.asarray(f_w1, F32), np.asarray(f_b1, F32),
                 np.asarray(f_w2, F32), np.asarray(f_b2, F32),
                 np.asarray(g_w1, F32), np.asarray(g_b1, F32),
                 np.asarray(g_w2, F32), np.asarray(g_b2, F32))
    return cores, groups, S_pad, packs, w


def _finalize(results, cores, x, g_w1, g_b1, g_w2, g_b2):
    """results: list of [64, NCW] per core -> full [N, 64] output."""
    out = np.empty((N, 64), dtype=F32)
    for c, core in enumerate(cores):
        out[core["order"] + c * NCN] = results[c][:, :NCN].T
    empties = np.concatenate([c["empty"] for c in cores])
    if empties.size:
        def celu(v):
            return np.maximum(v, 0) + np.minimum(0, np.expm1(np.minimum(v, 0)))
        u_in = np.concatenate(
            [np.zeros((empties.size, 64), F32), x[empties]], axis=1)
        u = celu(u_in @ g_w1 + g_b1)
        out[empties] = celu(u @ g_w2 + g_b2).astype(F32)
    return out


def kernel(x, pos, edge_index, f_w1, f_b1, f_w2, f_b2,
           g_w1, g_b1, g_w2, g_b2, _debug_numpy=False, _trace=False):
    x = np.asarray(x, F32)
    pos = np.asarray(pos, F32)
    cores, groups, S_pad, packs, w = _prepare(
        x, pos, edge_index, f_w1, f_b1, f_w2, f_b2, g_w1, g_b1, g_w2, g_b2)

    if _debug_numpy:
        results = [_numpy_device(f, xn, w, groups) for (f, xn) in packs]
        return _finalize(results, cores, x, np.asarray(g_w1, F32),
                         np.asarray(g_b1, F32), np.asarray(g_w2, F32),
                         np.asarray(g_b2, F32))

    _import_concourse()
    run_kwargs = {}
    if _trace:
        _install_ntff_shim()
        import concourse.bass_utils as _bu
        _bu.upload_artifacts = lambda tmpdir: f"file://{tmpdir}"
        import tempfile
        trace_dir = tempfile.mkdtemp(prefix="bass_trace_")
        run_kwargs = dict(tmpdir=trace_dir)
        kernel._last_trace_dir = trace_dir
    from concourse.bass_utils import run_bass_kernel_spmd

    nc = _build_nc(groups, S_pad)
    in_maps = [{"feats": feats, "xnode": xnode, "wpack": w["wpack"]}
               for (feats, xnode) in packs]
    res = run_bass_kernel_spmd(nc, in_maps, list(range(CORES)), trace=_trace,
                               **run_kwargs)
    results = [res.results[c]["out"] for c in range(CORES)]
    out = _finalize(results, cores, x, np.asarray(g_w1, F32),
                    np.asarray(g_b1, F32), np.asarray(g_w2, F32),
                    np.asarray(g_b2, F32))
    if _trace:
        kernel._last_exec_time_ns = res.exec_time_ns
        kernel._last_mean_exec_time_ns = res.mean_exec_time_ns
    return out


# revision 52
# speedup vs baseline: 1.9186x; 1.9186x over previous
"""Trainium2 Bass kernel for a GNN message-passing layer.

Reference semantics (per edge e = (src j, dst i)):
    m_in  = [x_j, pos_j - pos_i]                 # [E, 6]
    h     = celu(m_in @ f_w1 + f_b1)             # [E, 64]
    msg   = relu(h @ f_w2 + f_b2)                # [E, 64]
    aggr  = segment_max(msg, dst, N); empty -> 0 # [N, 64]
    u     = celu([aggr, x] @ g_w1 + g_b1)
    out   = celu(u @ g_w2 + g_b2)                # [N, 64]

Sharding: nodes are split into 8 contiguous ranges (6250 per core); each core
receives exactly the edges whose dst lands in its range, so the segment-max is
purely local (no collective).  The host does index-only work: it sorts each
core's nodes by in-degree, lays edges out in "rounds" (round r = the r-th edge
of every node that has one), pairs rounds two-high into 128-partition tiles,
and pads with duplicate edges (max is idempotent).  The device then does every
FLOP: the per-edge MLP as three accumulated matmul streams (celu decomposed as
celu(z) = z + relu(-z) + exp(-relu(-z)) - 1, with the linear z term re-streamed
through the combined weight W1@W2 and all constants folded into one bias), a
running tensor_max over the round tiles, and the node MLP with the same trick.
"""

import math
import os
import sys

import numpy as np

N = 50000
E = 1600000
CORES = 8
NCN = N // CORES            # nodes per core
TILE = 512                  # fp32 matmul moving free dim / one PSUM bank
GRP = 1024                  # uniform processing-group width (columns)
NCW = ((NCN + GRP - 1) // GRP) * GRP      # aggr width per core (7168)
SUP = 4096                  # feats DMA staging superblock (columns)
F32 = np.float32


# --------------------------------------------------------------------------
# host-side layout (index work only)
# --------------------------------------------------------------------------

def _core_layouts(edge_index):
    """Per-core node ordering + degree-sorted CSR of local edges."""
    dst = np.asarray(edge_index[1])
    cores = []
    for c in range(CORES):
        lo, hi = c * NCN, (c + 1) * NCN
        eids = np.nonzero((dst >= lo) & (dst < hi))[0]
        ldst = (dst[eids] - lo).astype(np.int64)
        deg = np.bincount(ldst, minlength=NCN)
        order = np.argsort(-deg, kind="stable")         # node ranks
        rank = np.empty(NCN, np.int64)
        rank[order] = np.arange(NCN)
        perm = np.argsort(rank[ldst], kind="stable")
        es = eids[perm]                                  # edges sorted by rank
        deg_s = deg[order]
        row_start = np.zeros(NCN + 1, np.int64)
        np.cumsum(deg_s, out=row_start[1:])
        cores.append(dict(es=es, deg_s=deg_s, row_start=row_start,
                          order=order, empty=order[deg_s == 0] + lo))
    return cores


def _tile_plan(cores):
    """Shared (SPMD-uniform) tile plan.

    All groups are uniform GRP (=1024) columns: matmul instructions carry at
    most ONE hardware sync wait, so every group needs "twin" matmuls with a
    free wait slot for redistributed dependencies (see _build_nc).

    Returns (tiles, groups, S):
      tiles  : list of (pair_round t, k) -- k-th 512-tile of pair-round t
      groups : list of (slot_col0, aggr_col0, fd, is_first_round)
      S      : total slot columns (= 512 * len(tiles))
    """
    rmax = max(int(c["deg_s"][0]) for c in cores)
    n_pairs = (rmax + 1) // 2
    tiles = []
    for t in range(n_pairs):
        w = max(int(np.searchsorted(-c["deg_s"], -(2 * t), side="left"))
                for c in cores)      # max over cores of #nodes with deg > 2t
        k_t = 2 * max(1, (w + GRP - 1) // GRP)       # 512-tiles, even count
        for k in range(k_t):
            tiles.append((t, k))
    S = TILE * len(tiles)

    groups = []
    for i in range(0, len(tiles), 2):
        t, k = tiles[i]
        groups.append((i * TILE, k * TILE, GRP, t == 0))
    return tiles, groups, S


def _pack_core(core, tiles, S, x, pos, src, dst):
    """Build one core's slot->edge assignment and gather features."""
    es, deg_s, row_start = core["es"], core["deg_s"], core["row_start"]
    ncols = len(tiles) * TILE
    nvec = np.tile(np.arange(TILE, dtype=np.int64), len(tiles))  # col in tile
    kvec = np.repeat([k for (_, k) in tiles], TILE)
    tvec = np.repeat([t for (t, _) in tiles], TILE)
    node = kvec * TILE + nvec                    # node rank targeted by column

    safe_node = np.minimum(node, NCN - 1)
    ecap = len(es) - 1
    first_edge = es[np.minimum(row_start[safe_node], ecap)]  # dup fallback
    # nodes with deg 0 or node>=NCN: fall back to edge es[0] (results ignored
    # or fixed up on host)
    bad = (node >= NCN) | (deg_s[safe_node] == 0)
    first_edge = np.where(bad, es[0], first_edge)

    def round_edges(r):
        has = (~bad) & (deg_s[safe_node] > r)
        idx = np.minimum(row_start[safe_node] + np.where(has, r, 0), ecap)
        return np.where(has, es[idx], first_edge)

    a_e = round_edges(2 * tvec)        # vectorized: r differs per column
    b_e = round_edges(2 * tvec + 1)

    feats = np.empty((18, S), dtype=F32)
    for half, eids in ((0, a_e), (9, b_e)):
        s, d = src[eids], dst[eids]
        feats[half + 0:half + 3, :ncols] = x[s].T
        feats[half + 3:half + 6, :ncols] = pos[s].T
        feats[half + 6:half + 9, :ncols] = pos[d].T
    if ncols < S:
        feats[:, ncols:] = 0.0

    xnode = np.zeros((3, NCW), dtype=F32)
    xnode[:, :NCN] = x[core["order"] + 0].T      # caller adds core offset
    return feats, xnode


# column layouts of the packed weight tensors: matmul operands go to a bf16
# pack (PE runs fp32 as ~4 internal passes; bf16 is ~4x faster with f32 PSUM
# accumulation), biases stay f32
WSLOTS = dict(w2=(128, 0, 128), w1n=(18, 128, 128), w12=(18, 256, 128),
              g1n=(67, 384, 64), g12=(67, 448, 64), g2=(64, 512, 64))
WCOL = 576
BSLOTS = dict(nbias1=(128, 0, 1), cbias=(64, 1, 1), nbias_g1=(64, 2, 1),
              nbias_gf=(64, 3, 1), pbias_gf=(64, 4, 1))
BCOL = 8


def _weights(f_w1, f_b1, f_w2, f_b2, g_w1, g_b1, g_w2, g_b2):
    w9 = np.concatenate([f_w1[0:3], f_w1[3:6], -f_w1[3:6]], axis=0)  # [9,64]
    blk = lambda m: np.block([[m, np.zeros_like(m)], [np.zeros_like(m), m]])
    w12 = (w9 @ f_w2).astype(F32)
    cbias = (f_b1 @ f_w2 - f_w2.sum(axis=0) + f_b2).astype(F32)       # [64]
    gbias = (g_b1 @ g_w2 - g_w2.sum(axis=0) + g_b2).astype(F32)       # [64]
    w = dict(
        w1n=blk(-w9).astype(F32),            # [18,128]
        w12=blk(w12),                        # [18,128]
        w2=blk(f_w2).astype(F32),            # [128,128]
        nbias1=np.tile(-f_b1, 2).astype(F32).reshape(128, 1),
        cbias=cbias.reshape(64, 1),
        g1n=(-g_w1).astype(F32),             # [67,64]
        g12=(g_w1 @ g_w2).astype(F32),       # [67,64]
        g2=g_w2.astype(F32),                 # [64,64]
        nbias_g1=(-g_b1).astype(F32).reshape(64, 1),
        nbias_gf=(-gbias).reshape(64, 1),
        pbias_gf=gbias.reshape(64, 1),
    )
    import ml_dtypes
    wpack = np.zeros((128, WCOL), dtype=ml_dtypes.bfloat16)
    for name, (p, c0, cn) in WSLOTS.items():
        wpack[:p, c0:c0 + cn] = w[name]
    bpack = np.zeros((128, BCOL), dtype=F32)
    for name, (p, c0, cn) in BSLOTS.items():
        bpack[:p, c0:c0 + cn] = w[name]
    w["wpack"] = wpack
    w["bpack"] = bpack
    return w


# --------------------------------------------------------------------------
# numpy model of the device program (for validation)
# --------------------------------------------------------------------------

def _numpy_device(feats, xnode, w, groups):
    aggr = np.empty((128, NCW), dtype=F32)
    for (c0, a0, fd, first) in groups:
        f = feats[:, c0:c0 + fd]
        zb = w["w1n"].T.astype(F32) @ f
        msg = w["w12"].T @ f
        r = np.maximum(zb + w["nbias1"], 0).astype(F32)
        e = np.exp(-r).astype(F32)
        msg = msg + w["w2"].T @ r + w["w2"].T @ e
        if first:
            aggr[:, a0:a0 + fd] = msg
        else:
            aggr[:, a0:a0 + fd] = np.maximum(aggr[:, a0:a0 + fd], msg)
    a64 = np.maximum(aggr[0:64], aggr[64:128])
    u_in = np.empty((67, NCW), dtype=F32)
    u_in[0:64] = np.maximum(a64 + w["cbias"], 0)
    u_in[64:67] = xnode
    zg = w["g1n"].T @ u_in
    rg = np.maximum(zg + w["nbias_g1"], 0).astype(F32)
    eg = np.exp(-rg).astype(F32)
    o2 = w["g12"].T @ u_in + w["g2"].T @ rg + w["g2"].T @ eg
    rf = np.maximum(-o2 + w["nbias_gf"], 0).astype(F32)
    ef = np.exp(-rf).astype(F32)
    vf = np.maximum(o2 + w["pbias_gf"], 0).astype(F32)
    return (vf - 1.0 + ef).astype(F32)        # [64, NCW]


# --------------------------------------------------------------------------
# bass program
# --------------------------------------------------------------------------

def _import_concourse():
    try:
        import concourse.bass  # noqa: F401
    except ImportError:
        sys.path.insert(0, "/opt/trn_rl_repo")


def _install_ntff_shim():
    """Provide antenv.axon_hooks (missing in this image) so that
    run_bass_kernel_spmd(trace=True) can capture NTFF profiles through
    libaxon's C ABI (same mechanism as trn_boot's degraded hook)."""
    import contextlib
    import ctypes
    import types

    if "antenv.axon_hooks" in sys.modules:
        return
    so_path = "/opt/axon/libaxon_pjrt.so"
    if not os.path.exists(so_path):
        return
    lib = ctypes.CDLL(so_path)
    if not hasattr(lib, "axon_start_nrt_profile"):
        return
    lib.axon_start_nrt_profile.argtypes = [ctypes.POINTER(ctypes.c_int64),
                                           ctypes.c_size_t]
    lib.axon_start_nrt_profile.restype = ctypes.c_int64
    lib.axon_stop_nrt_profile.argtypes = [ctypes.c_char_p]
    lib.axon_stop_nrt_profile.restype = ctypes.c_int64

    @contextlib.contextmanager
    def _hook(output_dir, device_ids):
        import jax
        jax.devices()
        if device_ids:
            ids = (ctypes.c_int64 * len(device_ids))(*device_ids)
            rc = lib.axon_start_nrt_profile(ids, len(device_ids))
        else:
            rc = lib.axon_start_nrt_profile(None, 0)
        if rc != 0:
            raise RuntimeError(f"axon_start_nrt_profile rc={rc}")
        try:
            yield
        finally:
            n = lib.axon_stop_nrt_profile(str(output_dir).encode())
            print(f"ntff profile: {n} file(s) -> {output_dir}",
                  file=sys.stderr)

    mod = types.ModuleType("antenv.axon_hooks")
    mod.get_axon_ntff_profile_hook = lambda: _hook
    mod.set_axon_ntff_profile_hook = lambda h: None
    sys.modules["antenv.axon_hooks"] = mod


def _dep(from_inst, to_inst, reason):
    from concourse.tile import add_dep_helper
    a = getattr(from_inst, "ins", from_inst)
    b = getattr(to_inst, "ins", to_inst)
    add_dep_helper(a, b, reason=reason)


def _build_nc(groups, S):
    _import_concourse()
    import concourse.bass as bass
    import concourse.tile as tile
    import concourse.tile_sem_assignment as _tsa
    from concourse import mybir

    # One DMAHW bookkeeping lane: HWDGE transfers then share a FIFO proc, so
    # DMA-vs-DMA ordering (slot WAW) needs no extra sync wait — ISA structs
    # carry at most one wait each.
    _tsa.NUM_HWDGE_SEMS = 1

    f32 = mybir.dt.float32
    bf16 = mybir.dt.bfloat16
    AF = mybir.ActivationFunctionType
    nc = bass.Bass()

    feats_d = nc.dram_tensor("feats", [18, S], bf16, kind="ExternalInput")
    xnode_d = nc.dram_tensor("xnode", [3, NCW], bf16, kind="ExternalInput")
    wpack_d = nc.dram_tensor("wpack", [128, WCOL], bf16, kind="ExternalInput")
    bpack_d = nc.dram_tensor("bpack", [128, BCOL], f32, kind="ExternalInput")
    out_d = nc.dram_tensor("out", [64, NCW], f32, kind="ExternalOutput")

    n_sup = (S + SUP - 1) // SUP

    with tile.TileContext(nc) as tc:
        with (
            tc.tile_pool(name="const", bufs=1) as cpool,
            tc.tile_pool(name="aggr", bufs=1) as apool,
            tc.tile_pool(name="feats", bufs=2) as fpool,
            tc.tile_pool(name="re", bufs=2) as repool,
            tc.tile_pool(name="gwork", bufs=1) as gpool,
            tc.tile_pool(name="psum_z", bufs=2, space="PSUM") as pz,
            tc.tile_pool(name="psum_m", bufs=2, space="PSUM") as pm,
        ):
            wsb = cpool.tile([128, WCOL], bf16, name="wsb")
            wdma = nc.sync.dma_start(wsb[:], wpack_d[:])
            bsb = cpool.tile([128, BCOL], f32, name="bsb")
            bdma = nc.sync.dma_start(bsb[:], bpack_d[:])
            w = {name: wsb[0:p, c0:c0 + cn]
                 for name, (p, c0, cn) in WSLOTS.items()}
            w.update({name: bsb[0:p, c0:c0 + cn]
                      for name, (p, c0, cn) in BSLOTS.items()})
            # ACT-side absorber: observe the bias DMA once so the first
            # bias-consuming activation doesn't need a second wait.
            tabs = cpool.tile([1, 1], f32, name="tabs")
            nc.scalar.activation(tabs[:], bsb[0:1, 0:1], AF.Copy)

            aggr = apool.tile([128, NCW], f32)

            # Matmult instructions can carry exactly one hardware sync wait;
            # a tiny absorber matmul observes the weights DMA so later
            # matmuls never need a second wait for it.
            scratch = pz.tile([128, GRP], f32, tag="zb", name="scratch")
            nc.tensor.matmul(scratch[0:1, 0:1], wsb[0:1, 0:1], wsb[0:1, 0:1],
                             start=True, stop=True)

            # Wait-absorber micro-ops: every ISA struct carries at most ONE
            # sync wait, so secondary dependencies are pre-observed by tiny
            # ops on the same engine/queue, ordered before the real op.
            vscr = cpool.tile([1, len(groups) + 4], f32, name="vscr")
            ascr = cpool.tile([1, NCW // TILE + 2], f32, name="ascr")
            ascr2 = cpool.tile([1, NCW // TILE + 2], f32, name="ascr2")
            # DVE-side absorber: observe the weights DMA once so DVE micro-
            # copies sourced from wsb need no DMA wait of their own.
            tvd0 = nc.vector.tensor_copy(vscr[0:1, len(groups) + 1:
                                              len(groups) + 2], bsb[0:1, 0:1])
            _dep(tvd0, bdma, "DVE observes bias DMA")

            sup_tiles = []
            sup_dmas = []
            for i in range(n_sup):
                cols = min(SUP, S - i * SUP)
                st = fpool.tile([18, SUP], bf16, tag="feats_sup")
                d = nc.sync.dma_start(st[:, :cols],
                                      feats_d[:, i * SUP:i * SUP + cols])
                sup_tiles.append(st)
                sup_dmas.append(d)

            runmax = []          # per-group reducer instruction
            for gi, (c0, a0, fd, first) in enumerate(groups):
                st = sup_tiles[c0 // SUP]
                fo = c0 % SUP
                fa = st[:, fo:fo + fd]
                zb = pz.tile([128, fd], f32, tag="zb")
                ms = pm.tile([128, fd], f32, tag="ms")
                mm_zb = [nc.tensor.matmul(zb[:, o:o + TILE], w["w1n"],
                                          fa[:, o:o + TILE],
                                          start=True, stop=True)
                         for o in range(0, fd, TILE)]
                # redistribute waits: the DVE release of this group's ms slot
                # lands on the second zb matmul (wait-free) instead of the
                # first ms matmul (which already carries a PE self-wait).
                if gi >= 2:
                    _dep(mm_zb[1], runmax[gi - 2], "ms-slot release via zb twin")
                # a new feats superblock must land before the NEXT group that
                # reads it; its wait goes on this group's e-twin (below).
                for o in range(0, fd, TILE):
                    nc.tensor.matmul(ms[:, o:o + TILE], w["w12"],
                                     fa[:, o:o + TILE], start=True, stop=False)
                r = repool.tile([128, fd], bf16, tag="r")
                e = repool.tile([128, fd], bf16, tag="e")
                nc.scalar.activation(r[:], zb[:], AF.Relu,
                                     bias=w["nbias1"], scale=1.0)
                nc.scalar.activation(e[:], r[:], AF.Exp, scale=-1.0)
                for o in range(0, fd, TILE):
                    nc.tensor.matmul(ms[:, o:o + TILE], w["w2"],
                                     r[:, o:o + TILE], start=False, stop=False)
                mm_e = [nc.tensor.matmul(ms[:, o:o + TILE], w["w2"],
                                         e[:, o:o + TILE],
                                         start=False, stop=(o + TILE >= fd))
                        for o in range(0, fd, TILE)]
                nxt = (c0 + fd) // SUP
                if nxt > c0 // SUP and nxt < n_sup:
                    _dep(mm_e[1], sup_dmas[nxt], "sup prefetch via e twin")
                # DVE pre-observes the msg matmuls' completion so the reducer
                # carries only its own in-order RAW wait.
                tv = nc.vector.tensor_copy(vscr[0:1, gi:gi + 1],
                                           bsb[0:1, 0:1])
                _dep(tv, mm_e[1], "absorb reducer PE wait")
                dst_ap = aggr[:, a0:a0 + fd]
                if first:
                    rm = nc.vector.tensor_copy(dst_ap, ms[:])
                else:
                    rm = nc.vector.tensor_max(dst_ap, dst_ap, ms[:])
                _dep(rm, tv, "order after absorber")
                runmax.append(rm)
                last_mm = mm_e[1]
                zb_last = zb

            # ---- node phase ----
            # TensorTensor needs equal base partitions for SBUF inputs:
            # DMA-move the odd-rounds half (partitions 64-127) down to 0-63.
            ah = gpool.tile([64, NCW], f32, tag="ah")
            ahdma = nc.sync.dma_start(ah[:], aggr[64:128, :])
            tva = nc.vector.tensor_copy(vscr[0:1, len(groups):len(groups) + 1],
                                        bsb[0:1, 0:1])
            _dep(tva, ahdma, "absorb aggr-move DMA wait")
            fold = nc.vector.tensor_max(ah[:], aggr[0:64, :], ah[:])
            _dep(fold, tva, "order after absorber")
            u_in = gpool.tile([67, NCW], bf16, tag="u_in")
            urelu = nc.scalar.activation(u_in[0:64, :], ah[:], AF.Relu,
                                         bias=w["cbias"], scale=1.0)
            xdma = nc.sync.dma_start(u_in[64:67, :], xnode_d[:])
            out_sb = gpool.tile([64, NCW], f32, tag="out_sb")

            # Absorber chain: tiny matmuls into the last group's dead zb
            # tile (claiming no new PSUM slot) make PE observe the final
            # reducer's DVE tick, the xnode DMA, and the u_in relu, so each
            # g-phase matmul keeps at most one hardware wait (its own PSUM
            # slot-reuse self-wait).
            scr2 = zb_last
            t2 = nc.tensor.matmul(scr2[0:1, 0:1], wsb[0:1, 0:1],
                                  wsb[0:1, 0:1], start=True, stop=False)
            _dep(t2, runmax[-1], "observe final reducer DVE tick")
            t3 = nc.tensor.matmul(scr2[0:1, 0:1], wsb[0:1, 0:1],
                                  wsb[0:1, 0:1], start=False, stop=False)
            _dep(t3, xdma, "observe xnode DMA")
            t4 = nc.tensor.matmul(scr2[0:1, 0:1], wsb[0:1, 0:1],
                                  wsb[0:1, 0:1], start=False, stop=True)
            _dep(t4, urelu, "observe u_in relu ACT tick")

            for i in range(NCW // TILE):
                ui = u_in[:, i * TILE:(i + 1) * TILE]
                zg = pz.tile([64, TILE], f32, tag="zb")
                o2 = pm.tile([64, TILE], f32, tag="ms")
                mm_zg = nc.tensor.matmul(zg[:], w["g1n"], ui,
                                         start=True, stop=True)
                nc.tensor.matmul(o2[:], w["g12"], ui, start=True, stop=False)
                rg = repool.tile([64, TILE], bf16, tag="r")
                eg = repool.tile([64, TILE], bf16, tag="e")
                # ACT pre-observes the g1 matmul so rg keeps only its own
                # slot-WAW wait
                tag_ = nc.scalar.activation(ascr2[0:1, i:i + 1], bsb[0:1, 0:1],
                                            AF.Copy)
                _dep(tag_, mm_zg, "absorb rg PE wait")
                rgi = nc.scalar.activation(rg[:], zg[:], AF.Relu,
                                           bias=w["nbias_g1"], scale=1.0)
                _dep(rgi, tag_, "order after absorber")
                nc.scalar.activation(eg[:], rg[:], AF.Exp, scale=-1.0)
                nc.tensor.matmul(o2[:], w["g2"], rg[:], start=False,
                                 stop=False)
                nc.tensor.matmul(o2[:], w["g2"], eg[:], start=False,
                                 stop=True)
                rf = repool.tile([64, TILE], f32, tag="rf")
                ef = repool.tile([64, TILE], f32, tag="ef")
                vf = repool.tile([64, TILE], f32, tag="vf")
                rf_act_deps = []
                if i >= 2:
                    # ACT pre-observes the combiner's DVE tick (releases the
                    # rf/ef/vf slots of tile i-2)
                    ta = nc.scalar.activation(ascr[0:1, i:i + 1],
                                              bsb[0:1, 0:1], AF.Copy)
                    _dep(ta, stt_prev2, "absorb final-combine DVE wait")
                    rf_act_deps.append(ta)
                rfi = nc.scalar.activation(rf[:], o2[:], AF.Relu,
                                           bias=w["nbias_gf"], scale=-1.0)
                for ta_ in rf_act_deps:
                    _dep(rfi, ta_, "order after absorber")
                nc.scalar.activation(ef[:], rf[:], AF.Exp, scale=-1.0)
                nc.scalar.activation(vf[:], o2[:], AF.Relu,
                                     bias=w["pbias_gf"], scale=1.0)
                stt = nc.vector.scalar_tensor_tensor(
                    out_sb[:, i * TILE:(i + 1) * TILE], vf[:], -1.0, ef[:],
                    op0=mybir.AluOpType.add, op1=mybir.AluOpType.add)
                if i >= 1:
                    stt_prev2 = stt_prev
                stt_prev = stt

            nc.sync.dma_start(out_d[:], out_sb[:])

    _prune_waits(nc)
    return nc


def _prune_waits(nc):
    """ISA structs carry at most one sync wait. Drop provably-redundant
    waits Tile emitted:

    1. same-engine self-waits on compute instructions other than Matmult:
       ACT/DVE/Pool queues are strict FIFO and each op fully drains before
       the next issues, so an earlier instruction on the same engine is
       always complete; the dependency the wait encodes is enforced by
       program order (the earlier instruction itself blocks the queue while
       ITS waits are pending).  PE kept: consecutive matmuls overlap
       fill/drain in the array.
    2. DMA-vs-DMA ordering waits on transfers that also carry a compute
       wait: in this program's dataflow the compute dependency is on
       readers of the slot's previous contents (or on consumers downstream
       of every earlier conflicting transfer), and a completed read implies
       the producing DMA completed.
    """
    n1 = n2 = 0
    for b in nc.m.functions[0].blocks:
        for i in b.instructions:
            si = i.sync_info
            if si is None or not si.on_wait or len(si.on_wait) < 2:
                continue
            nm = type(i).__name__
            waits = list(si.on_wait)
            if nm == "InstDrain":
                # kernel-tail drain: every engine's last instruction is
                # observed (transitively) by the final output DMA, so the
                # single DMAHW wait subsumes the engine waits here.
                dma_w = [x for x in waits if x.ant_name.startswith("DMAHW")]
                if dma_w:
                    si.on_wait = dma_w[-1:]
                else:
                    si.on_wait = waits[-1:]
                continue
            if nm == "InstDMACopy":
                if any(not x.ant_name.startswith("DMAHW") and
                       not x.ant_name.startswith("DMASW") for x in waits):
                    kept = [x for x in waits
                            if not (x.ant_name.startswith("DMAHW") or
                                    x.ant_name.startswith("DMASW"))]
                    n2 += len(waits) - len(kept)
                    waits = kept
            else:
                # Matmult included: matmuls complete in pc order (start AND
                # end monotone), and every PSUM slot-reuse WAW in this
                # program is >=8 matmuls distant, far beyond the fill/drain
                # overlap of adjacent instructions.
                own = str(i.engine).split(".")[-1]
                kept = [x for x in waits
                        if x.ant_name.rsplit("_", 1)[0] != own]
                if len(kept) < len(waits):
                    n1 += len(waits) - len(kept)
                    waits = kept
            si.on_wait = waits
    return n1, n2


# --------------------------------------------------------------------------
# entry points
# --------------------------------------------------------------------------

def _prepare(x, pos, edge_index, f_w1, f_b1, f_w2, f_b2,
             g_w1, g_b1, g_w2, g_b2):
    x = np.asarray(x, F32)
    pos = np.asarray(pos, F32)
    src = np.asarray(edge_index[0]).astype(np.int64)
    dst = np.asarray(edge_index[1]).astype(np.int64)
    cores = _core_layouts(edge_index)
    tiles, groups, S = _tile_plan(cores)
    S_pad = ((S + SUP - 1) // SUP) * SUP
    packs = []
    for c, core in enumerate(cores):
        feats, xnode = _pack_core(core, tiles, S_pad, x, pos, src, dst)
        xnode[:, :NCN] = x[core["order"] + c * NCN].T
        packs.append((feats, xnode))
    w = _weights(np.asarray(f_w1, F32), np.asarray(f_b1, F32),
                 np.asarray(f_w2, F32), np.asarray(f_b2, F32),
                 np.asarray(g_w1, F32), np.asarray(g_b1, F32),
                 np.asarray(g_w2, F32), np.asarray(g_b2, F32))
    return cores, groups, S_pad, packs, w


def _finalize(results, cores, x, g_w1, g_b1, g_w2, g_b2):
    """results: list of [64, NCW] per core -> full [N, 64] output."""
    out = np.empty((N, 64), dtype=F32)
    for c, core in enumerate(cores):
        out[core["order"] + c * NCN] = results[c][:, :NCN].T
    empties = np.concatenate([c["empty"] for c in cores])
    if empties.size:
        def celu(v):
            return np.maximum(v, 0) + np.minimum(0, np.expm1(np.minimum(v, 0)))
        u_in = np.concatenate(
            [np.zeros((empties.size, 64), F32), x[empties]], axis=1)
        u = celu(u_in @ g_w1 + g_b1)
        out[empties] = celu(u @ g_w2 + g_b2).astype(F32)
    return out


def kernel(x, pos, edge_index, f_w1, f_b1, f_w2, f_b2,
           g_w1, g_b1, g_w2, g_b2, _debug_numpy=False, _trace=False):
    x = np.asarray(x, F32)
    pos = np.asarray(pos, F32)
    cores, groups, S_pad, packs, w = _prepare(
        x, pos, edge_index, f_w1, f_b1, f_w2, f_b2, g_w1, g_b1, g_w2, g_b2)

    if _debug_numpy:
        results = [_numpy_device(f, xn, w, groups) for (f, xn) in packs]
        return _finalize(results, cores, x, np.asarray(g_w1, F32),
                         np.asarray(g_b1, F32), np.asarray(g_w2, F32),
                         np.asarray(g_b2, F32))

    _import_concourse()
    run_kwargs = {}
    if _trace:
        _install_ntff_shim()
        import concourse.bass_utils as _bu
        _bu.upload_artifacts = lambda tmpdir: f"file://{tmpdir}"
        import tempfile
        trace_dir = tempfile.mkdtemp(prefix="bass_trace_")
        run_kwargs = dict(tmpdir=trace_dir)
        kernel._last_trace_dir = trace_dir
    from concourse.bass_utils import run_bass_kernel_spmd

    import ml_dtypes
    bf = ml_dtypes.bfloat16
    nc = _build_nc(groups, S_pad)
    in_maps = [{"feats": feats.astype(bf), "xnode": xnode.astype(bf),
                "wpack": w["wpack"], "bpack": w["bpack"]}
               for (feats, xnode) in packs]
    res = run_bass_kernel_spmd(nc, in_maps, list(range(CORES)), trace=_trace,
                               **run_kwargs)
    results = [res.results[c]["out"] for c in range(CORES)]
    out = _finalize(results, cores, x, np.asarray(g_w1, F32),
                    np.asarray(g_b1, F32), np.asarray(g_w2, F32),
                    np.asarray(g_b2, F32))
    if _trace:
        kernel._last_exec_time_ns = res.exec_time_ns
        kernel._last_mean_exec_time_ns = res.mean_exec_time_ns
    return out
